# revision 4
# baseline (speedup 1.0000x reference)
"""Trainium2 Bass kernel for nn_CMPGNN_Net (gnn_message_passing), 8 NeuronCores.

Strategy (self-contained; hardcoded for N=10000, E=100000, G=64, F=64, D=256, L=3):
  - Nodes padded to 10240, sharded 1280/core. Edges bucketed by dst node-tile
    (128 nodes); every (core, tile) bucket is padded to a uniform Q edges so a
    single SPMD program covers all cores (Q adapts to the data).
  - cat([x_dst, x_src, gf[batch_dst]]) @ mg1_w is decomposed into per-node
    tables XDG = x@Wd + onehot_batch@(gf@Wg) + b1 (dst side, local) and
    XS = x@Ws (src side, all-gathered), fetched per-edge with dma_gather.
  - Edge MLP runs feature-major (PE transposes in/out); scatter-add to nodes
    is a one-hot matmul accumulated in PSUM (deterministic, race-free).
  - [G,D] readout partials + node-norm segment sums are all-gathered and
    summed on every core; the GRU runs redundantly on all cores.
  - Host (numpy): edge permutation/packing, classifier block, ggl, lgl
    un-permutation.
"""
import sys
sys.path.insert(0, '/opt/trn_rl_repo')

from contextlib import ExitStack

import numpy as np

import concourse.bacc as bacc
import concourse.tile as tile
from concourse import mybir
from concourse.bass_utils import run_bass_kernel_spmd
from concourse.masks import make_identity

# ---------------- problem constants ----------------
N, E, G = 10000, 100000, 64
F, D, L = 64, 256, 3
HID, NCLS = 256, 10
EPS, SLOPE = 1e-5, 0.01
NCORES = 8
NP = 10240
NLOC = NP // NCORES          # 1280
TPC = NLOC // 128            # 10 node tiles per core
P = 128
NRD = L + 1                  # 4 readouts

F32 = mybir.dt.float32
MM_DT = mybir.dt.float32     # matmul dtype knob: float32 | float32r
I16 = mybir.dt.int16

AF = mybir.ActivationFunctionType
OP = mybir.AluOpType

_cache = {}


def _pack_idx(idx, C):
    """Pack indices for dma_gather: [128, C//16], wrapped in 16 partitions and
    replicated across the 8 16-partition groups."""
    out = np.zeros((P, C // 16), np.int16)
    cols = np.arange(C) // 16
    rows = np.arange(C) % 16
    for g in range(8):
        out[16 * g + rows, cols] = idx
    return out


def _ts(i, n):
    return slice(i * n, (i + 1) * n)


def _build(Q):
    """Build the SPMD Bass program; structure depends only on Q."""
    C = TPC * Q               # per-core edge capacity
    SG = Q // 512             # 512-edge subgroups per node tile
    CH = C // 128             # 128-edge chunks

    nc = bacc.Bacc("TRN2", target_bir_lowering=False, debug=False,
                   num_devices=NCORES)

    def inp(name, shape, dt=F32):
        return nc.dram_tensor(name, shape, dt, kind="ExternalInput").ap()

    # shared weights
    fc_rhs = inp("fc_rhs", [F + 1, D])
    rdx0 = inp("rdx0", [F + 1, D])
    wd0 = inp("wd0", [F + 1, D])
    ws0 = inp("ws0", [F + 1, D])
    rdwx = inp("rdwx", [P, 2, D])
    rdwg = inp("rdwg", [P, 2, D])
    wd_all = inp("wd_all", [L, P, 2, D])
    ws_all = inp("ws_all", [L, P, 2, D])
    wgm_all = inp("wgm_all", [L, P, 2, D])
    w2_all = inp("w2_all", [L, P, 2, D])
    gate_all = inp("gate_all", [L, P, 2, D])
    b2c_all = inp("b2c_all", [L, P, 2])
    bgc_all = inp("bgc_all", [L, P, 2])
    bias_ext = inp("bias_ext", [G + 1, (L + 1) * D])   # row 64: rd_b, mg1_b[i]
    wih_in = inp("wih_t", [P, 2, 3 * D])
    whh_in = inp("whh_t", [P, 2, 3 * D])
    bsum_in = inp("bsum_row", [1, 3 * D])
    bih_in = inp("bih_row", [1, 3 * D])
    bhh_in = inp("bhh_row", [1, 3 * D])
    # per-core data
    x0t_in = inp("x0t", [F + 1, NLOC])
    ohnm_in = inp("ohnm", [P, TPC, G])
    oht_in = inp("oht", [G + 1, TPC, P])
    dst_gp = inp("dst_gp", [P, C // 16], I16)
    src_gp = inp("src_gp", [P, C // 16], I16)
    dstf_in = inp("dstf", [P, CH])
    iota_in = inp("iota", [P, P])

    out_gfnn = nc.dram_tensor("out_gfnn", [NRD, G, D + 1], F32,
                              kind="ExternalOutput").ap()
    out_gf = nc.dram_tensor("out_gf", [G, D], F32, kind="ExternalOutput").ap()
    out_lgl = nc.dram_tensor("out_lgl", [L, C], F32, kind="ExternalOutput").ap()

    with tile.TileContext(nc) as tc, ExitStack() as ctx:
        sbc = ctx.enter_context(tc.tile_pool(name="const", bufs=1))
        sbw = ctx.enter_context(tc.tile_pool(name="work", bufs=1))
        sbe = ctx.enter_context(tc.tile_pool(name="edge", bufs=2))
        psp = ctx.enter_context(tc.tile_pool(name="psp", bufs=2, space="PSUM"))
        drp = ctx.enter_context(tc.tile_pool(name="drp", bufs=2, space="DRAM"))

        # ---- constants: DMA -> rotating fp32 raw slot -> DVE cast ----
        def load_t(ap_in, shape, tag, dt=MM_DT, pool=sbc, bufs=1):
            raw = sbc.tile([P, 2, 3 * D], F32, tag="ldraw", bufs=2)
            rv = raw[:].rearrange("p a b -> p (a b)")[0:shape[0], 0:int(np.prod(shape[1:]))]
            if len(shape) == 3:
                rv = rv.rearrange("p (a b) -> p a b", a=shape[1])
            nc.sync.dma_start(out=rv, in_=ap_in)
            t = pool.tile(shape, dt, tag=tag, bufs=bufs)
            nc.vector.tensor_copy(t[:], rv)
            return t

        fc_rhs_t = load_t(fc_rhs, [F + 1, D], "fcr")
        rdx0_t = load_t(rdx0, [F + 1, D], "rdx0")
        wd0_t = load_t(wd0, [F + 1, D], "wd0")
        ws0_t = load_t(ws0, [F + 1, D], "ws0")
        rdwx_t = load_t(rdwx, [P, 2, D], "rdwx")
        rdwg_t = load_t(rdwg, [P, 2, D], "rdwg")
        ohnm_t = load_t(ohnm_in, [P, TPC, G], "ohnm")
        oht_t = load_t(oht_in, [G + 1, TPC, P], "oht")
        wih_t = load_t(wih_in, [P, 2, 3 * D], "wih")
        whh_t = load_t(whh_in, [P, 2, 3 * D], "whh")
        bsum_t = load_t(bsum_in, [1, 3 * D], "bsum")
        bih_t = load_t(bih_in, [1, 3 * D], "bih")
        bhh_t = load_t(bhh_in, [1, 3 * D], "bhh")
        bias_ext_t = load_t(bias_ext, [G + 1, (L + 1) * D], "biasext")
        iota_t = load_t(iota_in, [P, P], "iota", dt=F32)
        dstf_t = load_t(dstf_in, [P, CH], "dstf", dt=F32)

        w2b, gateb, b2c, bgc = [], [], [], []
        for i in range(L):
            w2b.append(load_t(w2_all[i], [P, 2, D], f"w2{i}"))
            gateb.append(load_t(gate_all[i], [P, 2, D], f"gate{i}"))
            b2c.append(load_t(b2c_all[i], [P, 2], f"b2c{i}", dt=F32))
            bgc.append(load_t(bgc_all[i], [P, 2], f"bgc{i}", dt=F32))

        ident_raw = sbc.tile([P, P], F32, tag="ident_r")
        make_identity(nc, ident_raw[:])
        ident_t = sbc.tile([P, P], MM_DT, tag="ident")
        nc.vector.tensor_copy(ident_t[:], ident_raw[:])
        ones_raw = sbc.tile([P, 1], F32, tag="ones_r")
        nc.gpsimd.memset(ones_raw[:], 1.0)
        onescol_t = sbc.tile([P, 1], MM_DT, tag="onescol")
        nc.vector.tensor_copy(onescol_t[:], ones_raw[:])
        ones64_t = sbc.tile([1, G], MM_DT, tag="ones64")
        nc.vector.tensor_copy(ones64_t[:], ones_raw[0:1, 0:1].to_broadcast([1, G]))
        zero_raw = sbc.tile([G, D], F32, tag="zero_r")
        nc.gpsimd.memset(zero_raw[:], 0.0)

        dstg_t = sbc.tile([P, C // 16], I16, tag="dstg")
        nc.sync.dma_start(out=dstg_t[:], in_=dst_gp)
        srcg_t = sbc.tile([P, C // 16], I16, tag="srcg")
        nc.sync.dma_start(out=srcg_t[:], in_=src_gp)

        # x0t shares the xT slot (dead before first xT is built)
        x0t_raw = sbw.tile([F + 1, NLOC], F32, tag="x0t_r")
        nc.sync.dma_start(out=x0t_raw[:], in_=x0t_in)
        x0t_t = sbw.tile([F + 1, NLOC], MM_DT, tag="xT",
                         padded_shape=[P, 2 * NLOC])
        nc.vector.tensor_copy(x0t_t[:], x0t_raw[:])

        # ---- initial state ----
        gf_t = sbw.tile([G, D], MM_DT, tag="gf", bufs=2)
        nc.vector.tensor_copy(gf_t[:], zero_raw[:])
        gff = sbw.tile([G, D], F32, tag="gff", bufs=2)
        nc.vector.tensor_copy(gff[:], zero_raw[:])
        state = {"gff": gff, "x_nm": None, "xT": None}

        # fc: x = x0 @ fc_w + fc_b
        x_nm = sbw.tile([P, TPC, D], MM_DT, tag="x_nm")
        for t in range(TPC):
            ps = psp.tile([P, D], F32, tag="pA", bufs=2, space="PSUM")
            nc.tensor.matmul(ps[:], x0t_t[:, _ts(t, P)], fc_rhs_t[:],
                             start=True, stop=True)
            nc.scalar.activation(x_nm[:, t, :], ps[:], AF.Copy)
        state["x_nm"] = x_nm

        def transpose_128(src_ap, dst_psum_ap):
            pin = src_ap.partition_size()
            nc.tensor.transpose(dst_psum_ap, src_ap, ident_t[0:pin, 0:pin])

        def transpose_gf(src, tag="gft"):
            out = sbw.tile([P, 2, G], MM_DT, tag=tag, bufs=3)
            for k in range(2):
                ps = psp.tile([P, G], MM_DT, tag="pA", bufs=2, space="PSUM")
                transpose_128(src[:, _ts(k, P)], ps[:])
                nc.vector.tensor_copy(out[:, k, :], ps[:])
            return out

        # ---------- readout ----------
        def readout(i, gfT):
            gfrd = sbw.tile([G + 1, D], MM_DT, tag="gfrd")
            ps = psp.tile([G, D], F32, tag="pA", bufs=2, space="PSUM")
            for k in range(2):
                nc.tensor.matmul(ps[:], gfT[:, k, :], rdwg_t[:, k, :],
                                 start=(k == 0), stop=(k == 1))
            nc.scalar.activation(gfrd[0:G, :], ps[:], AF.Copy)
            nc.vector.tensor_copy(gfrd[G:G + 1, :], bias_ext_t[G:G + 1, 0:D])

            seg_ps = psp.tile([G, D + 1], F32, tag="seg", bufs=1, space="PSUM")
            for t in range(TPC):
                gw_ps = psp.tile([P, D], F32, tag="pA", bufs=2, space="PSUM")
                if i == 0:
                    nc.tensor.matmul(gw_ps[:], x0t_t[:, _ts(t, P)], rdx0_t[:],
                                     start=True, stop=False)
                else:
                    for k in range(2):
                        nc.tensor.matmul(gw_ps[:], state["xT"][:, k, _ts(t, P)],
                                         rdwx_t[:, k, :], start=(k == 0),
                                         stop=False)
                nc.tensor.matmul(gw_ps[:], oht_t[:, t, :], gfrd[:],
                                 start=False, stop=True)
                gw = sbw.tile([P, D], F32, tag="gw", bufs=2)
                nc.scalar.activation(gw[:], gw_ps[:], AF.Sigmoid)
                seg_rhs = sbw.tile([P, D + 1], MM_DT, tag="segrhs", bufs=2)
                nc.vector.tensor_tensor(out=seg_rhs[:, 0:D], in0=gw[:],
                                        in1=state["x_nm"][:, t, :], op=OP.mult)
                junk = sbw.tile([P, D], F32, tag="junk")
                ssq = sbw.tile([P, 1], F32, tag="ssq", bufs=2)
                nc.scalar.activation(junk[:], gw[:], AF.Square,
                                     accum_out=ssq[:, 0:1])
                nc.scalar.activation(seg_rhs[:, D:D + 1], ssq[:, 0:1], AF.Sqrt)
                nc.tensor.matmul(seg_ps[:], ohnm_t[:, t, :], seg_rhs[:],
                                 start=(t == 0), stop=(t == TPC - 1),
                                 skip_group_check=True)
            part = sbw.tile([G, D + 1], F32, tag="part")
            nc.scalar.activation(part[:], seg_ps[:], AF.Copy)

            agi = drp.tile([G, D + 1], F32, tag="agi")
            ago = drp.tile([NCORES * G, D + 1], F32, tag="ago")
            nc.gpsimd.dma_start(out=agi[:], in_=part[:])
            nc.gpsimd.collective_compute(
                "AllGather", OP.bypass,
                replica_groups=[list(range(NCORES))],
                ins=[agi.opt()], outs=[ago.opt()])
            agg = sbw.tile([G, NCORES, D + 1], F32, tag="agg")
            nc.gpsimd.dma_start(
                out=agg[:], in_=ago[:].rearrange("(r g) n -> g r n", g=G))
            s1 = sbw.tile([G, 4, D + 1], F32, tag="s1")
            nc.vector.tensor_tensor(out=s1[:], in0=agg[:, 0:4, :],
                                    in1=agg[:, 4:8, :], op=OP.add)
            s2 = sbw.tile([G, 2, D + 1], F32, tag="s2")
            nc.vector.tensor_tensor(out=s2[:], in0=s1[:, 0:2, :],
                                    in1=s1[:, 2:4, :], op=OP.add)
            gfnn = sbw.tile([G, D + 1], F32, tag="gfnn")
            nc.vector.tensor_tensor(out=gfnn[:], in0=s2[:, 0, :],
                                    in1=s2[:, 1, :], op=OP.add)
            nc.sync.dma_start(out=out_gfnn[i], in_=gfnn[:])
            return gfnn

        # ---------- GRU (redundant on all cores) ----------
        def gru(gfnn, gfT_old):
            gfnew = sbw.tile([G, D], MM_DT, tag="gfnew")
            nc.vector.tensor_copy(gfnew[:], gfnn[:, 0:D])
            gfnT = transpose_gf(gfnew)

            def gate_sig(j):
                ps = psp.tile([G, D], F32, tag="pA", bufs=2, space="PSUM")
                for k in range(2):
                    nc.tensor.matmul(ps[:], gfnT[:, k, :], wih_t[:, k, _ts(j, D)],
                                     start=(k == 0), stop=False)
                for k in range(2):
                    nc.tensor.matmul(ps[:], gfT_old[:, k, :],
                                     whh_t[:, k, _ts(j, D)],
                                     start=False, stop=False)
                nc.tensor.matmul(ps[:], ones64_t[:], bsum_t[:, _ts(j, D)],
                                 start=False, stop=True)
                h = sbw.tile([G, D], F32, tag="grut", bufs=5)
                nc.scalar.activation(h[:], ps[:], AF.Sigmoid)
                return h

            r = gate_sig(0)
            z = gate_sig(1)

            inn_ps = psp.tile([G, D], F32, tag="pA", bufs=2, space="PSUM")
            for k in range(2):
                nc.tensor.matmul(inn_ps[:], gfnT[:, k, :], wih_t[:, k, _ts(2, D)],
                                 start=(k == 0), stop=False)
            nc.tensor.matmul(inn_ps[:], ones64_t[:], bih_t[:, _ts(2, D)],
                             start=False, stop=True)
            hn_ps = psp.tile([G, D], F32, tag="pA", bufs=2, space="PSUM")
            for k in range(2):
                nc.tensor.matmul(hn_ps[:], gfT_old[:, k, :], whh_t[:, k, _ts(2, D)],
                                 start=(k == 0), stop=False)
            nc.tensor.matmul(hn_ps[:], ones64_t[:], bhh_t[:, _ts(2, D)],
                             start=False, stop=True)
            hn = sbw.tile([G, D], F32, tag="grut", bufs=5)
            nc.scalar.activation(hn[:], hn_ps[:], AF.Copy)
            rhn = sbw.tile([G, D], F32, tag="grut", bufs=5)
            nc.vector.tensor_tensor(out=rhn[:], in0=r[:], in1=hn[:], op=OP.mult)
            nin = sbw.tile([G, D], F32, tag="grut", bufs=5)
            nc.vector.tensor_tensor(out=nin[:], in0=inn_ps[:], in1=rhn[:],
                                    op=OP.add)
            n_ = sbw.tile([G, D], F32, tag="grut", bufs=5)
            nc.scalar.activation(n_[:], nin[:], AF.Tanh)
            dif = sbw.tile([G, D], F32, tag="grut", bufs=5)
            nc.vector.tensor_tensor(out=dif[:], in0=state["gff"][:], in1=n_[:],
                                    op=OP.subtract)
            zd = sbw.tile([G, D], F32, tag="grut", bufs=5)
            nc.vector.tensor_tensor(out=zd[:], in0=z[:], in1=dif[:], op=OP.mult)
            gf_new = sbw.tile([G, D], MM_DT, tag="gf", bufs=2)
            nc.vector.tensor_tensor(out=gf_new[:], in0=n_[:], in1=zd[:],
                                    op=OP.add)
            gff2 = sbw.tile([G, D], F32, tag="gff", bufs=2)
            nc.vector.tensor_tensor(out=gff2[:], in0=n_[:], in1=zd[:], op=OP.add)
            state["gff"] = gff2
            return gf_new

        # ---------------- main loop ----------------
        for i in range(NRD):
            gfT = transpose_gf(gf_t)
            gfnn = readout(i, gfT)
            gf_t = gru(gfnn, gfT)
            if i == L:
                nc.sync.dma_start(out=out_gf, in_=gf_t[:].bitcast(F32))
                break

            # ---- per-node tables ----
            gfT2 = transpose_gf(gf_t)
            wgm_i = load_t(wgm_all[i], [P, 2, D], "wtmp", pool=sbw, bufs=3)
            wd_i = load_t(wd_all[i], [P, 2, D], "wtmp", pool=sbw, bufs=3)
            ws_i = load_t(ws_all[i], [P, 2, D], "wtmp", pool=sbw, bufs=3)
            gg = sbw.tile([G + 1, D], MM_DT, tag="gg")
            ps = psp.tile([G, D], F32, tag="pA", bufs=2, space="PSUM")
            for k in range(2):
                nc.tensor.matmul(ps[:], gfT2[:, k, :], wgm_i[:, k, :],
                                 start=(k == 0), stop=(k == 1))
            nc.scalar.activation(gg[0:G, :], ps[:], AF.Copy)
            nc.vector.tensor_copy(gg[G:G + 1, :],
                                  bias_ext_t[G:G + 1, _ts(i + 1, D)])

            xdg_dr = drp.tile([NLOC, D], F32, tag="xdg_dr")
            xs_dr = drp.tile([NLOC, D], F32, tag="xs_dr")
            for t in range(TPC):
                pd = psp.tile([P, D], F32, tag="pA", bufs=2, space="PSUM")
                if i == 0:
                    nc.tensor.matmul(pd[:], x0t_t[:, _ts(t, P)], wd0_t[:],
                                     start=True, stop=False)
                else:
                    for k in range(2):
                        nc.tensor.matmul(pd[:], state["xT"][:, k, _ts(t, P)],
                                         wd_i[:, k, :], start=(k == 0),
                                         stop=False)
                nc.tensor.matmul(pd[:], oht_t[:, t, :], gg[:], start=False,
                                 stop=True)
                st1 = sbw.tile([P, D], F32, tag="tbl", bufs=2)
                nc.scalar.activation(st1[:], pd[:], AF.Copy)
                nc.sync.dma_start(out=xdg_dr[_ts(t, P), :], in_=st1[:])
                pss = psp.tile([P, D], F32, tag="pA", bufs=2, space="PSUM")
                if i == 0:
                    nc.tensor.matmul(pss[:], x0t_t[:, _ts(t, P)], ws0_t[:],
                                     start=True, stop=True)
                else:
                    for k in range(2):
                        nc.tensor.matmul(pss[:], state["xT"][:, k, _ts(t, P)],
                                         ws_i[:, k, :], start=(k == 0),
                                         stop=(k == 1))
                st2 = sbw.tile([P, D], F32, tag="tbl", bufs=2)
                nc.scalar.activation(st2[:], pss[:], AF.Copy)
                nc.sync.dma_start(out=xs_dr[_ts(t, P), :], in_=st2[:])

            xsf_dr = drp.tile([NP, D], F32, tag="xsf_dr")
            nc.gpsimd.collective_compute(
                "AllGather", OP.bypass,
                replica_groups=[list(range(NCORES))],
                ins=[xs_dr.opt()], outs=[xsf_dr.opt()])

            # ---- edge phase ----
            x_new = sbw.tile([P, TPC, D], MM_DT, tag="x_nm")
            for t in range(TPC):
                xn_ps = psp.tile([P, D], F32, tag="xn", bufs=2, space="PSUM")
                for sg in range(SG):
                    e0 = t * Q + sg * 512
                    c0 = e0 // 128
                    gx = sbe.tile([P, 4, D], F32, tag="gxtt", bufs=2)
                    nc.gpsimd.dma_gather(gx[:], xdg_dr[:],
                                         dstg_t[:, e0 // 16:e0 // 16 + 32],
                                         512, 512, D)
                    gs = sbe.tile([P, 4, D], F32, tag="gsvals", bufs=2)
                    nc.gpsimd.dma_gather(gs[:], xsf_dr[:],
                                         srcg_t[:, e0 // 16:e0 // 16 + 32],
                                         512, 512, D)
                    pre1 = sbe.tile([P, 4, D], F32, tag="prevt", bufs=2)
                    nc.vector.tensor_tensor(out=pre1[:], in0=gx[:], in1=gs[:],
                                            op=OP.add)
                    a1 = sbe.tile([P, 4, D], MM_DT, tag="a1sqv", bufs=2)
                    nc.scalar.activation(a1[:], pre1[:], AF.Lrelu, alpha=SLOPE)
                    a1T = sbe.tile([P, 2, 512], MM_DT, tag="a1T", bufs=2)
                    for h in range(2):
                        tp = psp.tile([P, 512], MM_DT, tag="pA", bufs=2,
                                      space="PSUM")
                        for kk in range(4):
                            transpose_128(a1[:, kk, _ts(h, P)], tp[:, _ts(kk, P)])
                        if h == 0:
                            nc.vector.tensor_copy(a1T[:, h, :], tp[:])
                        else:
                            nc.scalar.activation(a1T[:, h, :], tp[:], AF.Copy)
                    msg = sbe.tile([P, 2, 512], MM_DT, tag="msg", bufs=2)
                    for m in range(2):
                        zp = psp.tile([P, 512], F32, tag="z512", bufs=2,
                                      space="PSUM")
                        for k in range(2):
                            nc.tensor.matmul(zp[:], w2b[i][:, k, _ts(m, P)],
                                             a1T[:, k, :], start=(k == 0),
                                             stop=(k == 1))
                        nc.scalar.activation(msg[:, m, :], zp[:], AF.Lrelu,
                                             bias=b2c[i][:, m:m + 1],
                                             alpha=SLOPE)
                    sig = sbe.tile([P, 2, 512], MM_DT, tag="sigoh", bufs=2)
                    for m in range(2):
                        lp = psp.tile([P, 512], F32, tag="z512", bufs=2,
                                      space="PSUM")
                        for k in range(2):
                            nc.tensor.matmul(lp[:], gateb[i][:, k, _ts(m, P)],
                                             msg[:, k, :], start=(k == 0),
                                             stop=(k == 1))
                        nc.scalar.activation(sig[:, m, :], lp[:], AF.Sigmoid,
                                             bias=bgc[i][:, m:m + 1])
                    tt = sbe.tile([P, 2, 512], MM_DT, tag="gxtt", bufs=2)
                    nc.vector.tensor_tensor(out=tt[:], in0=sig[:], in1=msg[:],
                                            op=OP.mult)
                    vals = sbe.tile([P, 2, 512], MM_DT, tag="gsvals", bufs=2)
                    nc.vector.tensor_tensor(out=vals[:], in0=tt[:], in1=msg[:],
                                            op=OP.mult)
                    sqv = sbe.tile([P, 2, 512], MM_DT, tag="a1sqv", bufs=2)
                    nc.scalar.activation(sqv[:], tt[:], AF.Square)
                    lg = psp.tile([1, 512], F32, tag="pA", bufs=2, space="PSUM")
                    for h in range(2):
                        nc.tensor.matmul(lg[:], onescol_t[:], sqv[:, h, :],
                                         start=(h == 0), stop=(h == 1))
                    lgst = sbe.tile([1, 512], F32, tag="lgst", bufs=2)
                    nc.scalar.activation(lgst[:], lg[:], AF.Sqrt)
                    nc.sync.dma_start(
                        out=out_lgl[i:i + 1, e0:e0 + 512], in_=lgst[:])
                    valsT = sbe.tile([P, 4, D], MM_DT, tag="prevt", bufs=2)
                    for half in range(2):
                        vp = psp.tile([P, 2, D], MM_DT, tag="pA", bufs=2,
                                      space="PSUM")
                        for kk in range(2):
                            ck = half * 2 + kk
                            for h in range(2):
                                transpose_128(vals[:, h, _ts(ck, P)],
                                              vp[:, kk, _ts(h, P)])
                        if half == 0:
                            nc.vector.tensor_copy(valsT[:, 0:2, :], vp[:])
                        else:
                            nc.scalar.activation(valsT[:, 2:4, :], vp[:],
                                                 AF.Copy)
                    for kk in range(4):
                        ck = c0 + kk
                        oh = sbe.tile([P, P], MM_DT, tag="sigoh", bufs=2)
                        nc.vector.tensor_scalar(
                            out=oh[:], in0=iota_t[:],
                            scalar1=dstf_t[:, ck:ck + 1],
                            scalar2=float(-128 * t),
                            op0=OP.subtract, op1=OP.is_equal)
                        nc.tensor.matmul(xn_ps[:], oh[:], valsT[:, kk, :],
                                         start=(sg == 0 and kk == 0),
                                         stop=(sg == SG - 1 and kk == 3),
                                         skip_group_check=True)
                nc.vector.tensor_copy(x_new[:, t, :], xn_ps[:])
            state["x_nm"] = x_new
            xT = sbw.tile([P, 2, NLOC], MM_DT, tag="xT")
            for t in range(TPC):
                tp = psp.tile([P, 2, P], MM_DT, tag="pA", bufs=2, space="PSUM")
                for h in range(2):
                    transpose_128(x_new[:, t, _ts(h, P)], tp[:, h, :])
                nc.scalar.activation(xT[:, 0, _ts(t, P)], tp[:, 0, :], AF.Copy)
                nc.vector.tensor_copy(xT[:, 1, _ts(t, P)], tp[:, 1, :])
            state["xT"] = xT

    nc.compile()
    return nc


def _host_prep(inputs):
    x = np.asarray(inputs["x"], np.float32)
    ei = np.asarray(inputs["edge_index"]).astype(np.int64)
    batch = np.asarray(inputs["batch"]).astype(np.int64)
    src, dst = ei[0], ei[1]

    w = {k: np.asarray(v, np.float32) for k, v in inputs.items()
         if k not in ("x", "edge_index", "batch")}

    # --- edge bucketing by dst node tile ---
    bin_id = dst // 128
    order = np.argsort(bin_id, kind="stable")
    counts = np.bincount(bin_id, minlength=80)[:80]
    Q = int(max(512, ((counts.max() + 511) // 512) * 512))
    C = TPC * Q
    CH = C // 128
    starts = np.zeros(81, np.int64)
    starts[1:] = np.cumsum(counts)

    per_core = []
    for c in range(NCORES):
        dst_loc_f = np.full(C, -1.0, np.float32)
        dst_gidx = np.zeros(C, np.int64)
        src_gidx = np.zeros(C, np.int64)
        origin = np.full(C, -1, np.int64)
        for t in range(TPC):
            b = c * TPC + t
            seg = order[starts[b]:starts[b + 1]]
            n = len(seg)
            o = t * Q
            dst_loc_f[o:o + n] = (dst[seg] - c * NLOC).astype(np.float32)
            dst_gidx[o:o + n] = dst[seg] - c * NLOC
            src_gidx[o:o + n] = src[seg]
            origin[o:o + n] = seg
        per_core.append(dict(
            dst_gp=_pack_idx(dst_gidx, C), src_gp=_pack_idx(src_gidx, C),
            dstf=np.ascontiguousarray(dst_loc_f.reshape(CH, P).T),
            origin=origin))

    # --- weights packing ---
    fc_w, fc_b = w["fc_w"], w["fc_b"]
    rd_w, rd_b = w["rd_w"], w["rd_b"]
    rdwx, rdwg = rd_w[:D], rd_w[D:]
    mg1_w, mg1_b = w["mg1_w"], w["mg1_b"]
    Wd, Ws, Wgm = mg1_w[:, :D, :], mg1_w[:, D:2 * D, :], mg1_w[:, 2 * D:, :]

    def pack2(a):  # [256, Dout] -> [128, 2, Dout]
        return np.ascontiguousarray(a.reshape(2, P, -1).transpose(1, 0, 2))

    bias_ext = np.zeros((G + 1, (L + 1) * D), np.float32)
    bias_ext[G, 0:D] = rd_b
    for i in range(L):
        bias_ext[G, _ts(i + 1, D)] = mg1_b[i]

    shared = dict(
        fc_rhs=np.concatenate([fc_w, fc_b[None]], 0),
        rdx0=np.concatenate([fc_w @ rdwx, (fc_b @ rdwx)[None]], 0),
        wd0=np.concatenate([fc_w @ Wd[0], (fc_b @ Wd[0])[None]], 0),
        ws0=np.concatenate([fc_w @ Ws[0], (fc_b @ Ws[0])[None]], 0),
        rdwx=pack2(rdwx), rdwg=pack2(rdwg),
        wd_all=np.stack([pack2(Wd[i]) for i in range(L)]),
        ws_all=np.stack([pack2(Ws[i]) for i in range(L)]),
        wgm_all=np.stack([pack2(Wgm[i]) for i in range(L)]),
        w2_all=np.stack([pack2(w["mg2_w"][i]) for i in range(L)]),
        gate_all=np.stack([pack2(w["gate_w"][i]) for i in range(L)]),
        b2c_all=np.ascontiguousarray(
            w["mg2_b"].reshape(L, 2, P).transpose(0, 2, 1)),
        bgc_all=np.ascontiguousarray(
            w["gate_b"].reshape(L, 2, P).transpose(0, 2, 1)),
        bias_ext=bias_ext,
        wih_t=pack2(np.ascontiguousarray(w["gru_wih"].T)),
        whh_t=pack2(np.ascontiguousarray(w["gru_whh"].T)),
        bsum_row=(w["gru_bih"] + w["gru_bhh"])[None, :],
        bih_row=np.ascontiguousarray(w["gru_bih"][None, :]),
        bhh_row=np.ascontiguousarray(w["gru_bhh"][None, :]),
        iota=np.tile(np.arange(P, dtype=np.float32)[None, :], (P, 1)),
    )

    x_pad = np.zeros((NP, F), np.float32)
    x_pad[:N] = x
    oh_full = np.zeros((NP, G), np.float32)
    oh_full[np.arange(N), batch] = 1.0
    for c in range(NCORES):
        xl = x_pad[c * NLOC:(c + 1) * NLOC]
        x0t = np.concatenate([xl.T, np.ones((1, NLOC), np.float32)], 0)
        ohl = oh_full[c * NLOC:(c + 1) * NLOC]
        ohnm = np.ascontiguousarray(ohl.reshape(TPC, P, G).transpose(1, 0, 2))
        oht = np.concatenate([ohl.T, np.ones((1, NLOC), np.float32)], 0)
        oht = np.ascontiguousarray(oht.reshape(G + 1, TPC, P))
        per_core[c].update(x0t=np.ascontiguousarray(x0t), ohnm=ohnm, oht=oht,
                           **shared)
    return per_core, Q, C, w, batch


def kernel(**inputs):
    per_core, Q, C, w, batch = _host_prep(inputs)

    if Q not in _cache:
        _cache[Q] = _build(Q)
    nc = _cache[Q]

    in_maps = [{k: v for k, v in pc.items() if k != "origin"}
               for pc in per_core]
    res = run_bass_kernel_spmd(nc, in_maps, core_ids=list(range(NCORES)))

    r0 = res.results[0]
    gf = r0["out_gf"]
    gfnn = r0["out_gfnn"]

    cnt = np.bincount(batch, minlength=G).astype(np.float32)
    nnseg = gfnn[:, :, D]
    ggl = (nnseg / np.maximum(cnt, 1.0)[None, :]).mean(axis=1).astype(np.float32)

    lgl = np.zeros((E, L), np.float32)
    for c in range(NCORES):
        dev = res.results[c]["out_lgl"]
        orig = per_core[c]["origin"]
        m = orig >= 0
        lgl[orig[m]] = dev[:, m].T

    mu = gf.mean(0)
    var = ((gf - mu) ** 2).mean(0)
    xn = (gf - mu) / np.sqrt(var + EPS) * w["bn_g"] + w["bn_b"]
    h1 = xn @ w["clf1_w"] + w["clf1_b"]
    h1 = np.where(h1 > 0, h1, SLOPE * h1)
    logits = h1 @ w["clf2_w"] + w["clf2_b"]
    logits = logits - logits.max(-1, keepdims=True)
    logits = (logits - np.log(np.exp(logits).sum(-1, keepdims=True))
              ).astype(np.float32)

    return logits, lgl, ggl


# revision 10
# speedup vs baseline: 1.0046x; 1.0046x over previous
"""Trainium2 Bass kernel for nn_CMPGNN_Net (gnn_message_passing), 8 NeuronCores.

Strategy (self-contained; hardcoded for N=10000, E=100000, G=64, F=64, D=256, L=3):
  - Nodes padded to 10240, sharded 1280/core. Edges bucketed by dst node-tile
    (128 nodes); every (core, tile) bucket is padded to a uniform Q edges so a
    single SPMD program covers all cores (Q adapts to the data).
  - cat([x_dst, x_src, gf[batch_dst]]) @ mg1_w is decomposed into per-node
    tables XDG = x@Wd + onehot_batch@(gf@Wg) + b1 (dst side, local) and
    XS = x@Ws (src side, all-gathered), fetched per-edge with dma_gather.
  - Edge MLP runs feature-major (PE transposes in/out); scatter-add to nodes
    is a one-hot matmul accumulated in PSUM (deterministic, race-free).
  - [G,D] readout partials + node-norm segment sums are all-gathered and
    summed on every core; the GRU runs redundantly on all cores.
  - Host (numpy): edge permutation/packing, classifier block, ggl, lgl
    un-permutation.
"""
import sys
sys.path.insert(0, '/opt/trn_rl_repo')

from contextlib import ExitStack

import numpy as np

import concourse.bacc as bacc
import concourse.tile as tile
from concourse import mybir
from concourse.bass_utils import run_bass_kernel_spmd
from concourse.masks import make_identity

# ---------------- problem constants ----------------
N, E, G = 10000, 100000, 64
F, D, L = 64, 256, 3
HID, NCLS = 256, 10
EPS, SLOPE = 1e-5, 0.01
NCORES = 8
NP = 10240
NLOC = NP // NCORES          # 1280
TPC = NLOC // 128            # 10 node tiles per core
P = 128
NRD = L + 1                  # 4 readouts

F32 = mybir.dt.float32
MM_DT = mybir.dt.float32     # matmul dtype knob: float32 | float32r
I16 = mybir.dt.int16

AF = mybir.ActivationFunctionType
OP = mybir.AluOpType

_cache = {}


def _pack_idx(idx, C):
    """Pack indices for dma_gather: [128, C//16], wrapped in 16 partitions and
    replicated across the 8 16-partition groups."""
    out = np.zeros((P, C // 16), np.int16)
    cols = np.arange(C) // 16
    rows = np.arange(C) % 16
    for g in range(8):
        out[16 * g + rows, cols] = idx
    return out


def _ts(i, n):
    return slice(i * n, (i + 1) * n)


def _build(Q):
    """Build the SPMD Bass program; structure depends only on Q."""
    C = TPC * Q               # per-core edge capacity
    SG = Q // 512             # 512-edge subgroups per node tile
    CH = C // 128             # 128-edge chunks

    nc = bacc.Bacc("TRN2", target_bir_lowering=False, debug=False,
                   num_devices=NCORES)

    def inp(name, shape, dt=F32):
        return nc.dram_tensor(name, shape, dt, kind="ExternalInput").ap()

    # shared weights
    fc_rhs = inp("fc_rhs", [F + 1, D])
    rdx0 = inp("rdx0", [F + 1, D])
    wd0 = inp("wd0", [F + 1, D])
    ws0 = inp("ws0", [F + 1, D])
    rdwx = inp("rdwx", [P, 2, D])
    rdwg = inp("rdwg", [P, 2, D])
    wd_all = inp("wd_all", [L, P, 2, D])
    ws_all = inp("ws_all", [L, P, 2, D])
    wgm_all = inp("wgm_all", [L, P, 2, D])
    w2_all = inp("w2_all", [L, P, 2, D])
    gate_all = inp("gate_all", [L, P, 2, D])
    b2c_all = inp("b2c_all", [L, P, 2])
    bgc_all = inp("bgc_all", [L, P, 2])
    bias_ext = inp("bias_ext", [G + 1, (L + 1) * D])   # row 64: rd_b, mg1_b[i]
    wih_in = inp("wih_t", [P, 2, 3 * D])
    whh_in = inp("whh_t", [P, 2, 3 * D])
    bsum_in = inp("bsum_row", [1, 3 * D])
    bih_in = inp("bih_row", [1, 3 * D])
    bhh_in = inp("bhh_row", [1, 3 * D])
    # per-core data
    x0t_in = inp("x0t", [F + 1, NLOC])
    ohnm_in = inp("ohnm", [P, TPC, G])
    oht_in = inp("oht", [G + 1, TPC, P])
    dst_gp = inp("dst_gp", [P, C // 16], I16)
    src_gp = inp("src_gp", [P, C // 16], I16)
    dstf_in = inp("dstf", [P, CH])
    iota_in = inp("iota", [P, P])

    out_gfnn = nc.dram_tensor("out_gfnn", [NRD, G, D + 1], F32,
                              kind="ExternalOutput").ap()
    out_gf = nc.dram_tensor("out_gf", [G, D], F32, kind="ExternalOutput").ap()
    out_lgl = nc.dram_tensor("out_lgl", [L, C], F32, kind="ExternalOutput").ap()

    with tile.TileContext(nc) as tc, ExitStack() as ctx:
        sbc = ctx.enter_context(tc.tile_pool(name="const", bufs=1))
        sbw = ctx.enter_context(tc.tile_pool(name="work", bufs=1))
        sbe = ctx.enter_context(tc.tile_pool(name="edge", bufs=2))
        psp = ctx.enter_context(tc.tile_pool(name="psp", bufs=2, space="PSUM"))
        drp = ctx.enter_context(tc.tile_pool(name="drp", bufs=2, space="DRAM"))

        # ---- constants: DMA -> rotating fp32 raw slot -> DVE cast ----
        def load_t(ap_in, shape, tag, dt=MM_DT, pool=sbc, bufs=1):
            raw = sbc.tile([P, 2, 3 * D], F32, tag="ldraw", bufs=1)
            rv = raw[:].rearrange("p a b -> p (a b)")[0:shape[0], 0:int(np.prod(shape[1:]))]
            if len(shape) == 3:
                rv = rv.rearrange("p (a b) -> p a b", a=shape[1])
            nc.sync.dma_start(out=rv, in_=ap_in)
            t = pool.tile(shape, dt, tag=tag, bufs=bufs)
            nc.vector.tensor_copy(t[:], rv)
            return t

        fc_rhs_t = load_t(fc_rhs, [F + 1, D], "fcr")
        rdx0_t = load_t(rdx0, [F + 1, D], "rdx0")
        wd0_t = load_t(wd0, [F + 1, D], "wd0")
        ws0_t = load_t(ws0, [F + 1, D], "ws0")
        rdwx_t = load_t(rdwx, [P, 2, D], "rdwx")
        rdwg_t = load_t(rdwg, [P, 2, D], "rdwg")
        ohnm_t = load_t(ohnm_in, [P, TPC, G], "ohnm")
        ohnm_f = load_t(ohnm_in, [P, TPC, G], "ohnmf", dt=F32)
        oht_t = load_t(oht_in, [G + 1, TPC, P], "oht")
        wih_t = load_t(wih_in, [P, 2, 3 * D], "wih")
        whh_t = load_t(whh_in, [P, 2, 3 * D], "whh")
        bsum_t = load_t(bsum_in, [1, 3 * D], "bsum")
        bih_t = load_t(bih_in, [1, 3 * D], "bih")
        bhh_t = load_t(bhh_in, [1, 3 * D], "bhh")
        bias_ext_t = load_t(bias_ext, [G + 1, (L + 1) * D], "biasext")
        iota_t = load_t(iota_in, [P, P], "iota", dt=F32)
        dstf_t = load_t(dstf_in, [P, CH], "dstf", dt=F32)

        w2b, gateb, b2c, bgc = [], [], [], []
        for i in range(L):
            w2b.append(load_t(w2_all[i], [P, 2, D], f"w2{i}"))
            gateb.append(load_t(gate_all[i], [P, 2, D], f"gate{i}"))
            b2c.append(load_t(b2c_all[i], [P, 2], f"b2c{i}", dt=F32))
            bgc.append(load_t(bgc_all[i], [P, 2], f"bgc{i}", dt=F32))

        ident_raw = sbc.tile([P, P], F32, tag="ident_r")
        make_identity(nc, ident_raw[:])
        ident_t = sbc.tile([P, P], MM_DT, tag="ident")
        nc.vector.tensor_copy(ident_t[:], ident_raw[:])
        ones_raw = sbc.tile([P, 1], F32, tag="ones_r")
        nc.gpsimd.memset(ones_raw[:], 1.0)
        onescol_t = sbc.tile([P, 1], MM_DT, tag="onescol")
        nc.vector.tensor_copy(onescol_t[:], ones_raw[:])
        ones64_t = sbc.tile([1, G], MM_DT, tag="ones64")
        nc.vector.tensor_copy(ones64_t[:], ones_raw[0:1, 0:1].to_broadcast([1, G]))
        zero_raw = sbc.tile([G, D], F32, tag="zero_r")
        nc.gpsimd.memset(zero_raw[:], 0.0)

        dstg_t = sbc.tile([P, C // 16], I16, tag="dstg")
        nc.sync.dma_start(out=dstg_t[:], in_=dst_gp)
        srcg_t = sbc.tile([P, C // 16], I16, tag="srcg")
        nc.sync.dma_start(out=srcg_t[:], in_=src_gp)

        # x0t shares the xT slot (dead before first xT is built)
        x0t_raw = sbw.tile([F + 1, NLOC], F32, tag="x0t_r")
        nc.sync.dma_start(out=x0t_raw[:], in_=x0t_in)
        x0t_t = sbw.tile([F + 1, NLOC], MM_DT, tag="xT",
                         padded_shape=[P, 2 * NLOC])
        nc.vector.tensor_copy(x0t_t[:], x0t_raw[:])

        # ---- initial state ----
        gf_t = sbw.tile([G, D], MM_DT, tag="gf", bufs=2)
        nc.vector.tensor_copy(gf_t[:], zero_raw[:])
        gff = sbw.tile([G, D], F32, tag="gff", bufs=2)
        nc.vector.tensor_copy(gff[:], zero_raw[:])
        state = {"gff": gff, "x_nm": None, "xT": None}

        # fc: x = x0 @ fc_w + fc_b
        x_nm = sbw.tile([P, TPC, D], MM_DT, tag="x_nm")
        for t in range(TPC):
            ps = psp.tile([P, D], F32, tag="pA", bufs=2, space="PSUM")
            nc.tensor.matmul(ps[:], x0t_t[:, _ts(t, P)], fc_rhs_t[:],
                             start=True, stop=True)
            nc.scalar.activation(x_nm[:, t, :], ps[:], AF.Copy)
        state["x_nm"] = x_nm

        def transpose_128(src_ap, dst_psum_ap):
            pin = src_ap.partition_size()
            nc.tensor.transpose(dst_psum_ap, src_ap, ident_t[0:pin, 0:pin])

        def transpose_gf(src, tag="gft"):
            out = sbw.tile([P, 2, G], MM_DT, tag=tag, bufs=3)
            for k in range(2):
                ps = psp.tile([P, G], MM_DT, tag="pA", bufs=2, space="PSUM")
                transpose_128(src[:, _ts(k, P)], ps[:])
                nc.vector.tensor_copy(out[:, k, :], ps[:])
            return out

        # ---------- readout ----------
        def readout(i, gfT):
            gfrd = sbw.tile([G + 1, D], MM_DT, tag="gfrd")
            ps = psp.tile([G, D], F32, tag="pA", bufs=2, space="PSUM")
            for k in range(2):
                nc.tensor.matmul(ps[:], gfT[:, k, :], rdwg_t[:, k, :],
                                 start=(k == 0), stop=(k == 1))
            nc.scalar.activation(gfrd[0:G, :], ps[:], AF.Copy)
            nc.vector.tensor_copy(gfrd[G:G + 1, :], bias_ext_t[G:G + 1, 0:D])

            seg_ps = psp.tile([G, D + 1], F32, tag="seg", bufs=1, space="PSUM")
            ssqcol = sbw.tile([P, TPC], F32, tag="ssqcol")
            for t in range(TPC):
                gw_ps = psp.tile([P, D], F32, tag="pA", bufs=2, space="PSUM")
                if i == 0:
                    nc.tensor.matmul(gw_ps[:], x0t_t[:, _ts(t, P)], rdx0_t[:],
                                     start=True, stop=False)
                else:
                    for k in range(2):
                        nc.tensor.matmul(gw_ps[:], state["xT"][:, k, _ts(t, P)],
                                         rdwx_t[:, k, :], start=(k == 0),
                                         stop=False)
                nc.tensor.matmul(gw_ps[:], oht_t[:, t, :], gfrd[:],
                                 start=False, stop=True)
                gw = sbw.tile([P, D], F32, tag="gw", bufs=2)
                nc.scalar.activation(gw[:], gw_ps[:], AF.Sigmoid)
                seg_rhs = sbw.tile([P, D], MM_DT, tag="segrhs", bufs=2)
                nc.vector.tensor_tensor(out=seg_rhs[:], in0=gw[:],
                                        in1=state["x_nm"][:, t, :], op=OP.mult)
                junk = sbw.tile([P, D], F32, tag="junk")
                nc.scalar.activation(junk[:], gw[:], AF.Square,
                                     accum_out=ssqcol[:, t:t + 1])
                nc.tensor.matmul(seg_ps[:, 0:D], ohnm_t[:, t, :], seg_rhs[:],
                                 start=(t == 0), stop=(t == TPC - 1),
                                 skip_group_check=True)
            nnall = sbw.tile([P, TPC], F32, tag="nnall")
            nc.scalar.activation(nnall[:], ssqcol[:], AF.Sqrt)
            for t in range(TPC):
                nc.tensor.matmul(seg_ps[:, D:D + 1], ohnm_f[:, t, :],
                                 nnall[:, t:t + 1],
                                 start=(t == 0), stop=(t == TPC - 1),
                                 skip_group_check=True)
            part = sbw.tile([G, D + 1], F32, tag="part")
            nc.scalar.activation(part[:], seg_ps[:], AF.Copy)

            agi = drp.tile([G, D + 1], F32, tag="agi")
            ago = drp.tile([NCORES * G, D + 1], F32, tag="ago",
                           addr_space="Shared")
            nc.sync.dma_start(out=agi[:], in_=part[:])
            nc.gpsimd.collective_compute(
                "AllGather", OP.bypass,
                replica_groups=[list(range(NCORES))],
                ins=[agi.opt()], outs=[ago.opt()])
            agg = sbw.tile([G, NCORES, D + 1], F32, tag="agg")
            nc.sync.dma_start(
                out=agg[:], in_=ago[:].rearrange("(r g) n -> g r n", g=G))
            s1 = sbw.tile([G, 4, D + 1], F32, tag="s1")
            nc.vector.tensor_tensor(out=s1[:], in0=agg[:, 0:4, :],
                                    in1=agg[:, 4:8, :], op=OP.add)
            s2 = sbw.tile([G, 2, D + 1], F32, tag="s2")
            nc.vector.tensor_tensor(out=s2[:], in0=s1[:, 0:2, :],
                                    in1=s1[:, 2:4, :], op=OP.add)
            gfnn = sbw.tile([G, D + 1], F32, tag="gfnn")
            nc.vector.tensor_tensor(out=gfnn[:], in0=s2[:, 0, :],
                                    in1=s2[:, 1, :], op=OP.add)
            nc.sync.dma_start(out=out_gfnn[i], in_=gfnn[:])
            return gfnn

        # ---------- GRU (redundant on all cores) ----------
        def gru(gfnn, gfT_old):
            gfnew = sbw.tile([G, D], MM_DT, tag="gfnew")
            nc.vector.tensor_copy(gfnew[:], gfnn[:, 0:D])
            gfnT = transpose_gf(gfnew)

            def gate_sig(j):
                ps = psp.tile([G, D], F32, tag="pA", bufs=2, space="PSUM")
                for k in range(2):
                    nc.tensor.matmul(ps[:], gfnT[:, k, :], wih_t[:, k, _ts(j, D)],
                                     start=(k == 0), stop=False)
                for k in range(2):
                    nc.tensor.matmul(ps[:], gfT_old[:, k, :],
                                     whh_t[:, k, _ts(j, D)],
                                     start=False, stop=False)
                nc.tensor.matmul(ps[:], ones64_t[:], bsum_t[:, _ts(j, D)],
                                 start=False, stop=True)
                h = sbw.tile([G, D], F32, tag="grut", bufs=5)
                nc.scalar.activation(h[:], ps[:], AF.Sigmoid)
                return h

            r = gate_sig(0)
            z = gate_sig(1)

            inn_ps = psp.tile([G, D], F32, tag="pA", bufs=2, space="PSUM")
            for k in range(2):
                nc.tensor.matmul(inn_ps[:], gfnT[:, k, :], wih_t[:, k, _ts(2, D)],
                                 start=(k == 0), stop=False)
            nc.tensor.matmul(inn_ps[:], ones64_t[:], bih_t[:, _ts(2, D)],
                             start=False, stop=True)
            hn_ps = psp.tile([G, D], F32, tag="pA", bufs=2, space="PSUM")
            for k in range(2):
                nc.tensor.matmul(hn_ps[:], gfT_old[:, k, :], whh_t[:, k, _ts(2, D)],
                                 start=(k == 0), stop=False)
            nc.tensor.matmul(hn_ps[:], ones64_t[:], bhh_t[:, _ts(2, D)],
                             start=False, stop=True)
            hn = sbw.tile([G, D], F32, tag="grut", bufs=5)
            nc.scalar.activation(hn[:], hn_ps[:], AF.Copy)
            rhn = sbw.tile([G, D], F32, tag="grut", bufs=5)
            nc.vector.tensor_tensor(out=rhn[:], in0=r[:], in1=hn[:], op=OP.mult)
            nin = sbw.tile([G, D], F32, tag="grut", bufs=5)
            nc.vector.tensor_tensor(out=nin[:], in0=inn_ps[:], in1=rhn[:],
                                    op=OP.add)
            n_ = sbw.tile([G, D], F32, tag="grut", bufs=5)
            nc.scalar.activation(n_[:], nin[:], AF.Tanh)
            dif = sbw.tile([G, D], F32, tag="grut", bufs=5)
            nc.vector.tensor_tensor(out=dif[:], in0=state["gff"][:], in1=n_[:],
                                    op=OP.subtract)
            zd = sbw.tile([G, D], F32, tag="grut", bufs=5)
            nc.vector.tensor_tensor(out=zd[:], in0=z[:], in1=dif[:], op=OP.mult)
            gf_new = sbw.tile([G, D], MM_DT, tag="gf", bufs=2)
            nc.vector.tensor_tensor(out=gf_new[:], in0=n_[:], in1=zd[:],
                                    op=OP.add)
            gff2 = sbw.tile([G, D], F32, tag="gff", bufs=2)
            nc.vector.tensor_tensor(out=gff2[:], in0=n_[:], in1=zd[:], op=OP.add)
            state["gff"] = gff2
            return gf_new

        # ---------------- main loop ----------------
        for i in range(NRD):
            # ---- XS table + AllGather first: overlaps the readout ----
            if i < L:
                ws_i = load_t(ws_all[i], [P, 2, D], "wtmp", pool=sbw, bufs=3)
                xs_dr = drp.tile([NLOC, D], F32, tag="xs_dr")
                for t in range(TPC):
                    pss = psp.tile([P, D], F32, tag="pA", bufs=2, space="PSUM")
                    if i == 0:
                        nc.tensor.matmul(pss[:], x0t_t[:, _ts(t, P)], ws0_t[:],
                                         start=True, stop=True)
                    else:
                        for k in range(2):
                            nc.tensor.matmul(pss[:], state["xT"][:, k, _ts(t, P)],
                                             ws_i[:, k, :], start=(k == 0),
                                             stop=(k == 1))
                    st2 = sbw.tile([P, D], F32, tag="tbl", bufs=2)
                    nc.scalar.activation(st2[:], pss[:], AF.Copy)
                    nc.sync.dma_start(out=xs_dr[_ts(t, P), :], in_=st2[:])
                xsf_dr = drp.tile([NP, D], F32, tag="xsf_dr",
                                  addr_space="Shared")
                nc.gpsimd.collective_compute(
                    "AllGather", OP.bypass,
                    replica_groups=[list(range(NCORES))],
                    ins=[xs_dr.opt()], outs=[xsf_dr.opt()])

            gfT = transpose_gf(gf_t)
            gfnn = readout(i, gfT)
            gf_t = gru(gfnn, gfT)
            if i == L:
                nc.sync.dma_start(out=out_gf, in_=gf_t[:].bitcast(F32))
                break

            # ---- XDG table (needs post-GRU gf) ----
            gfT2 = transpose_gf(gf_t)
            wgm_i = load_t(wgm_all[i], [P, 2, D], "wtmp", pool=sbw, bufs=3)
            wd_i = load_t(wd_all[i], [P, 2, D], "wtmp", pool=sbw, bufs=3)
            gg = sbw.tile([G + 1, D], MM_DT, tag="gg")
            ps = psp.tile([G, D], F32, tag="pA", bufs=2, space="PSUM")
            for k in range(2):
                nc.tensor.matmul(ps[:], gfT2[:, k, :], wgm_i[:, k, :],
                                 start=(k == 0), stop=(k == 1))
            nc.scalar.activation(gg[0:G, :], ps[:], AF.Copy)
            nc.vector.tensor_copy(gg[G:G + 1, :],
                                  bias_ext_t[G:G + 1, _ts(i + 1, D)])

            xdg_dr = drp.tile([NLOC, D], F32, tag="xdg_dr")
            for t in range(TPC):
                pd = psp.tile([P, D], F32, tag="pA", bufs=2, space="PSUM")
                if i == 0:
                    nc.tensor.matmul(pd[:], x0t_t[:, _ts(t, P)], wd0_t[:],
                                     start=True, stop=False)
                else:
                    for k in range(2):
                        nc.tensor.matmul(pd[:], state["xT"][:, k, _ts(t, P)],
                                         wd_i[:, k, :], start=(k == 0),
                                         stop=False)
                nc.tensor.matmul(pd[:], oht_t[:, t, :], gg[:], start=False,
                                 stop=True)
                st1 = sbw.tile([P, D], F32, tag="tbl", bufs=2)
                nc.scalar.activation(st1[:], pd[:], AF.Copy)
                nc.sync.dma_start(out=xdg_dr[_ts(t, P), :], in_=st1[:])

            # ---- edge phase ----
            x_new = sbw.tile([P, TPC, D], MM_DT, tag="x_nm")
            for t in range(TPC):
                xn_ps = psp.tile([P, D], F32, tag="xn", bufs=2, space="PSUM")
                lgst = sbe.tile([1, Q], F32, tag="lgst", bufs=2)
                for sg in range(SG):
                    e0 = t * Q + sg * 512
                    c0 = e0 // 128
                    gx = sbe.tile([P, 4, D], F32, tag="gxtt", bufs=2)
                    nc.gpsimd.dma_gather(gx[:], xdg_dr[:],
                                         dstg_t[:, e0 // 16:e0 // 16 + 32],
                                         512, 512, D)
                    gs = sbe.tile([P, 4, D], F32, tag="gsvals", bufs=2)
                    nc.gpsimd.dma_gather(gs[:], xsf_dr[:],
                                         srcg_t[:, e0 // 16:e0 // 16 + 32],
                                         512, 512, D)
                    pre1 = sbe.tile([P, 4, D], F32, tag="prevt", bufs=2)
                    nc.vector.tensor_tensor(out=pre1[:], in0=gx[:], in1=gs[:],
                                            op=OP.add)
                    a1 = sbe.tile([P, 4, D], MM_DT, tag="a1sqv", bufs=2)
                    nc.scalar.activation(a1[:], pre1[:], AF.Prelu, alpha=SLOPE)
                    a1T = sbe.tile([P, 2, 512], MM_DT, tag="a1T", bufs=2)
                    for h in range(2):
                        tp = psp.tile([P, 512], MM_DT, tag="pA", bufs=2,
                                      space="PSUM")
                        for kk in range(4):
                            transpose_128(a1[:, kk, _ts(h, P)], tp[:, _ts(kk, P)])
                        if h == 0:
                            nc.vector.tensor_copy(a1T[:, h, :], tp[:])
                        else:
                            nc.scalar.activation(a1T[:, h, :], tp[:], AF.Copy)
                    msg = sbe.tile([P, 2, 512], MM_DT, tag="msg", bufs=2)
                    for m in range(2):
                        zp = psp.tile([P, 512], F32, tag="z512", bufs=2,
                                      space="PSUM")
                        for k in range(2):
                            nc.tensor.matmul(zp[:], w2b[i][:, k, _ts(m, P)],
                                             a1T[:, k, :], start=(k == 0),
                                             stop=(k == 1))
                        nc.scalar.activation(msg[:, m, :], zp[:], AF.Prelu,
                                             bias=b2c[i][:, m:m + 1],
                                             alpha=SLOPE)
                    sig = sbe.tile([P, 2, 512], MM_DT, tag="sigoh", bufs=2)
                    for m in range(2):
                        lp = psp.tile([P, 512], F32, tag="z512", bufs=2,
                                      space="PSUM")
                        for k in range(2):
                            nc.tensor.matmul(lp[:], gateb[i][:, k, _ts(m, P)],
                                             msg[:, k, :], start=(k == 0),
                                             stop=(k == 1))
                        nc.scalar.activation(sig[:, m, :], lp[:], AF.Sigmoid,
                                             bias=bgc[i][:, m:m + 1])
                    tt = sbe.tile([P, 2, 512], MM_DT, tag="gxtt", bufs=2)
                    nc.vector.tensor_tensor(out=tt[:], in0=sig[:], in1=msg[:],
                                            op=OP.mult)
                    vals = sbe.tile([P, 2, 512], MM_DT, tag="gsvals", bufs=2)
                    nc.vector.tensor_tensor(out=vals[:], in0=tt[:], in1=msg[:],
                                            op=OP.mult)
                    sqv = sbe.tile([P, 2, 512], MM_DT, tag="a1sqv", bufs=2)
                    nc.scalar.activation(sqv[:], tt[:], AF.Square)
                    lg = psp.tile([1, 512], F32, tag="pA", bufs=2, space="PSUM")
                    for h in range(2):
                        nc.tensor.matmul(lg[:], onescol_t[:], sqv[:, h, :],
                                         start=(h == 0), stop=(h == 1))
                    nc.scalar.activation(lgst[:, _ts(sg, 512)], lg[:], AF.Copy)
                    valsT = sbe.tile([P, 4, D], MM_DT, tag="prevt", bufs=2)
                    for half in range(2):
                        vp = psp.tile([P, 2, D], MM_DT, tag="pA", bufs=2,
                                      space="PSUM")
                        for kk in range(2):
                            ck = half * 2 + kk
                            for h in range(2):
                                transpose_128(vals[:, h, _ts(ck, P)],
                                              vp[:, kk, _ts(h, P)])
                        if half == 0:
                            nc.vector.tensor_copy(valsT[:, 0:2, :], vp[:])
                        else:
                            nc.scalar.activation(valsT[:, 2:4, :], vp[:],
                                                 AF.Copy)
                    for kk in range(4):
                        ck = c0 + kk
                        oh = sbe.tile([P, P], MM_DT, tag="sigoh", bufs=2)
                        nc.vector.tensor_scalar(
                            out=oh[:], in0=iota_t[:],
                            scalar1=dstf_t[:, ck:ck + 1],
                            scalar2=float(-128 * t),
                            op0=OP.subtract, op1=OP.is_equal)
                        nc.tensor.matmul(xn_ps[:], oh[:], valsT[:, kk, :],
                                         start=(sg == 0 and kk == 0),
                                         stop=(sg == SG - 1 and kk == 3),
                                         skip_group_check=True)
                nc.vector.tensor_copy(x_new[:, t, :], xn_ps[:])
                nc.sync.dma_start(out=out_lgl[i:i + 1, _ts(t, Q)], in_=lgst[:])
            state["x_nm"] = x_new
            xT = sbw.tile([P, 2, NLOC], MM_DT, tag="xT")
            for t in range(TPC):
                tp = psp.tile([P, 2, P], MM_DT, tag="pA", bufs=2, space="PSUM")
                for h in range(2):
                    transpose_128(x_new[:, t, _ts(h, P)], tp[:, h, :])
                nc.scalar.activation(xT[:, 0, _ts(t, P)], tp[:, 0, :], AF.Copy)
                nc.vector.tensor_copy(xT[:, 1, _ts(t, P)], tp[:, 1, :])
            state["xT"] = xT

    nc.compile()
    return nc


def _host_prep(inputs):
    x = np.asarray(inputs["x"], np.float32)
    ei = np.asarray(inputs["edge_index"]).astype(np.int64)
    batch = np.asarray(inputs["batch"]).astype(np.int64)
    src, dst = ei[0], ei[1]

    w = {k: np.asarray(v, np.float32) for k, v in inputs.items()
         if k not in ("x", "edge_index", "batch")}

    # --- edge bucketing by dst node tile ---
    bin_id = dst // 128
    order = np.argsort(bin_id, kind="stable")
    counts = np.bincount(bin_id, minlength=80)[:80]
    Q = int(max(512, ((counts.max() + 511) // 512) * 512))
    C = TPC * Q
    CH = C // 128
    starts = np.zeros(81, np.int64)
    starts[1:] = np.cumsum(counts)

    per_core = []
    for c in range(NCORES):
        dst_loc_f = np.full(C, -1.0, np.float32)
        dst_gidx = np.zeros(C, np.int64)
        src_gidx = np.zeros(C, np.int64)
        origin = np.full(C, -1, np.int64)
        for t in range(TPC):
            b = c * TPC + t
            seg = order[starts[b]:starts[b + 1]]
            n = len(seg)
            o = t * Q
            dst_loc_f[o:o + n] = (dst[seg] - c * NLOC).astype(np.float32)
            dst_gidx[o:o + n] = dst[seg] - c * NLOC
            src_gidx[o:o + n] = src[seg]
            origin[o:o + n] = seg
        per_core.append(dict(
            dst_gp=_pack_idx(dst_gidx, C), src_gp=_pack_idx(src_gidx, C),
            dstf=np.ascontiguousarray(dst_loc_f.reshape(CH, P).T),
            origin=origin))

    # --- weights packing ---
    fc_w, fc_b = w["fc_w"], w["fc_b"]
    rd_w, rd_b = w["rd_w"], w["rd_b"]
    rdwx, rdwg = rd_w[:D], rd_w[D:]
    mg1_w, mg1_b = w["mg1_w"], w["mg1_b"]
    Wd, Ws, Wgm = mg1_w[:, :D, :], mg1_w[:, D:2 * D, :], mg1_w[:, 2 * D:, :]

    def pack2(a):  # [256, Dout] -> [128, 2, Dout]
        return np.ascontiguousarray(a.reshape(2, P, -1).transpose(1, 0, 2))

    bias_ext = np.zeros((G + 1, (L + 1) * D), np.float32)
    bias_ext[G, 0:D] = rd_b
    for i in range(L):
        bias_ext[G, _ts(i + 1, D)] = mg1_b[i]

    shared = dict(
        fc_rhs=np.concatenate([fc_w, fc_b[None]], 0),
        rdx0=np.concatenate([fc_w @ rdwx, (fc_b @ rdwx)[None]], 0),
        wd0=np.concatenate([fc_w @ Wd[0], (fc_b @ Wd[0])[None]], 0),
        ws0=np.concatenate([fc_w @ Ws[0], (fc_b @ Ws[0])[None]], 0),
        rdwx=pack2(rdwx), rdwg=pack2(rdwg),
        wd_all=np.stack([pack2(Wd[i]) for i in range(L)]),
        ws_all=np.stack([pack2(Ws[i]) for i in range(L)]),
        wgm_all=np.stack([pack2(Wgm[i]) for i in range(L)]),
        w2_all=np.stack([pack2(w["mg2_w"][i]) for i in range(L)]),
        gate_all=np.stack([pack2(w["gate_w"][i]) for i in range(L)]),
        b2c_all=np.ascontiguousarray(
            w["mg2_b"].reshape(L, 2, P).transpose(0, 2, 1)),
        bgc_all=np.ascontiguousarray(
            w["gate_b"].reshape(L, 2, P).transpose(0, 2, 1)),
        bias_ext=bias_ext,
        wih_t=pack2(np.ascontiguousarray(w["gru_wih"].T)),
        whh_t=pack2(np.ascontiguousarray(w["gru_whh"].T)),
        bsum_row=(w["gru_bih"] + w["gru_bhh"])[None, :],
        bih_row=np.ascontiguousarray(w["gru_bih"][None, :]),
        bhh_row=np.ascontiguousarray(w["gru_bhh"][None, :]),
        iota=np.tile(np.arange(P, dtype=np.float32)[None, :], (P, 1)),
    )

    x_pad = np.zeros((NP, F), np.float32)
    x_pad[:N] = x
    oh_full = np.zeros((NP, G), np.float32)
    oh_full[np.arange(N), batch] = 1.0
    for c in range(NCORES):
        xl = x_pad[c * NLOC:(c + 1) * NLOC]
        x0t = np.concatenate([xl.T, np.ones((1, NLOC), np.float32)], 0)
        ohl = oh_full[c * NLOC:(c + 1) * NLOC]
        ohnm = np.ascontiguousarray(ohl.reshape(TPC, P, G).transpose(1, 0, 2))
        oht = np.concatenate([ohl.T, np.ones((1, NLOC), np.float32)], 0)
        oht = np.ascontiguousarray(oht.reshape(G + 1, TPC, P))
        per_core[c].update(x0t=np.ascontiguousarray(x0t), ohnm=ohnm, oht=oht,
                           **shared)
    return per_core, Q, C, w, batch


def kernel(**inputs):
    per_core, Q, C, w, batch = _host_prep(inputs)

    if Q not in _cache:
        _cache[Q] = _build(Q)
    nc = _cache[Q]

    in_maps = [{k: v for k, v in pc.items() if k != "origin"}
               for pc in per_core]
    res = run_bass_kernel_spmd(nc, in_maps, core_ids=list(range(NCORES)))

    r0 = res.results[0]
    gf = r0["out_gf"]
    gfnn = r0["out_gfnn"]

    cnt = np.bincount(batch, minlength=G).astype(np.float32)
    nnseg = gfnn[:, :, D]
    ggl = (nnseg / np.maximum(cnt, 1.0)[None, :]).mean(axis=1).astype(np.float32)

    lgl = np.zeros((E, L), np.float32)
    for c in range(NCORES):
        dev = np.sqrt(np.maximum(res.results[c]["out_lgl"], 0.0))
        orig = per_core[c]["origin"]
        m = orig >= 0
        lgl[orig[m]] = dev[:, m].T

    mu = gf.mean(0)
    var = ((gf - mu) ** 2).mean(0)
    xn = (gf - mu) / np.sqrt(var + EPS) * w["bn_g"] + w["bn_b"]
    h1 = xn @ w["clf1_w"] + w["clf1_b"]
    h1 = np.where(h1 > 0, h1, SLOPE * h1)
    logits = h1 @ w["clf2_w"] + w["clf2_b"]
    logits = logits - logits.max(-1, keepdims=True)
    logits = (logits - np.log(np.exp(logits).sum(-1, keepdims=True))
              ).astype(np.float32)

    return logits, lgl, ggl
